# revision 2
# baseline (speedup 1.0000x reference)
"""Trainium2 Bass kernel for nn_DualModalExpertContainer.

Strategy (8 NeuronCores, data-parallel over batch, 4 batches/core):
  - expert0/1 depthwise 3x3: 9 accumulating fp32r diagonal matmuls on the
    TensorEngine over a host-padded 66x66 image (N=512 chunks, PSUM accum).
  - pointwise convs (128->256) and expert2 1x1 (256->256): fp32r matmuls.
  - BN+SiLU fused on the Scalar engine (per-partition scale/bias APs).
  - routing combine sum_e coeff[b,e]*silu_e on the Vector engine via
    tensor_scalar + 2 fused scalar_tensor_tensor ops.
  - host precomputes: padded x, folded BN scale/bias, routing coeffs,
    transposed/diagonal weight tiles.
"""
import sys

sys.path.insert(0, '/opt/trn_rl_repo')

import numpy as np

BN_EPS = 1e-5
B, C, H, W, OUT = 32, 256, 64, 64, 256
CS = C // 2                      # 128 channels per expert half
NCORES = 8
BL = B // NCORES                 # 4 batches per core
HP, WP = H + 2, W + 2            # 66x66 padded
SP = HP * WP                     # 4356
S = H * W                        # 4096
CH = 8                           # h-rows per chunk
NCK = H // CH                    # 8 chunks per image
NCOL = CH * W                    # 512 columns per chunk

# packed fp32r weight tile column offsets
DW_OFF = 0                       # 2 experts * 9 taps * 128 = 2304
PW_OFF = 2304                    # 2 experts * 256 = 512
W2_OFF = 2816                    # 2 k-halves * 256 = 512
NWR = 3328

_cache = {}


def _build_program():
    import concourse.bass as bass
    import concourse.mybir as mybir
    from concourse.bacc import Bacc
    from concourse import tile

    f32 = mybir.dt.float32
    f32r = mybir.dt.float32r
    SILU = mybir.ActivationFunctionType.Silu
    MUL = mybir.AluOpType.mult
    ADD = mybir.AluOpType.add

    nc = Bacc()
    xp_in = nc.declare_dram_parameter("xp", [BL * 2, CS, SP], f32r, isOutput=False)
    wr_in = nc.declare_dram_parameter("wr", [CS, NWR], f32r, isOutput=False)
    cs_in = nc.declare_dram_parameter("cs", [CS, 24], f32, isOutput=False)
    out_d = nc.declare_dram_parameter("out", [BL, OUT, S], f32, isOutput=True)

    with tile.TileContext(nc) as tc:
        with (
            tc.tile_pool(name="const", bufs=1) as cpool,
            tc.tile_pool(name="xpad", bufs=4) as xpool,
            tc.tile_pool(name="ysb", bufs=2) as ypool,
            tc.tile_pool(name="esb", bufs=2) as epool,
            tc.tile_pool(name="accs", bufs=3) as apool,
            tc.tile_pool(name="psy", bufs=2, space="PSUM") as psy_pool,
            tc.tile_pool(name="pse", bufs=1, space="PSUM") as pse_pool,
        ):
            wr = cpool.tile([CS, NWR], f32r)
            nc.sync.dma_start(wr[:], wr_in.ap())
            cs = cpool.tile([CS, 24], f32)
            nc.sync.dma_start(cs[:], cs_in.ap())

            def dw_lhs(e, t):
                off = DW_OFF + (e * 9 + t) * CS
                return wr[:, off:off + CS]

            def pw_lhs(e, h):
                off = PW_OFF + e * 256 + h * CS
                return wr[:, off:off + CS]

            def w2_lhs(kh, h):
                off = W2_OFF + kh * 256 + h * CS
                return wr[:, off:off + CS]

            def bn_scale(e, h):
                return cs[:, 2 * e + h:2 * e + h + 1]

            def bn_bias(e, h):
                return cs[:, 6 + 2 * e + h:6 + 2 * e + h + 1]

            def coeff(b, e):
                return cs[:, 12 + 3 * b + e:12 + 3 * b + e + 1]

            for b in range(BL):
                xpad = [None, None]
                for e in range(2):
                    t = xpool.tile([CS, HP, WP], f32r, tag=f"xp{e}")
                    nc.sync.dma_start(
                        t[:].rearrange("p h w -> p (h w)"), xp_in.ap()[2 * b + e])
                    xpad[e] = t

                # software-pipelined: DW for chunk n overlaps PW/e2/silu/
                # combine for chunk n-1
                ysb = {}
                for n in range(NCK + 1):
                    if n < NCK:
                        h0 = n * CH
                        for e in range(2):
                            ps = psy_pool.tile([CS, NCOL], f32, tag=f"y{e}")
                            psv = ps[:].rearrange("p (h w) -> p h w", w=W)
                            for t in range(9):
                                dh, dw = t // 3, t % 3
                                rhs = xpad[e][:, h0 + dh:h0 + dh + CH, dw:dw + W]
                                nc.tensor.matmul(psv, dw_lhs(e, t), rhs,
                                                 start=(t == 0), stop=(t == 8))
                            y = ypool.tile([CS, NCOL], f32r, tag=f"ysb{e}")
                            nc.scalar.copy(y[:], ps[:])
                            ysb[(n, e)] = y
                    if n >= 1:
                        m = n - 1
                        h0 = m * CH
                        es = {}
                        # pointwise experts 0/1
                        for e in range(2):
                            yt = ysb.pop((m, e))
                            for h in range(2):
                                pp = pse_pool.tile([CS, NCOL], f32, tag=f"pw{h}")
                                nc.tensor.matmul(pp[:], pw_lhs(e, h), yt[:],
                                                 start=True, stop=True)
                                s = epool.tile([CS, NCOL], f32, tag=f"es{e}{h}")
                                nc.scalar.activation(s[:], pp[:], SILU,
                                                     bias=bn_bias(e, h),
                                                     scale=bn_scale(e, h))
                                es[(e, h)] = s
                        # expert 2: 1x1 over all 256 channels
                        for h in range(2):
                            pp = pse_pool.tile([CS, NCOL], f32, tag=f"e2{h}")
                            for kh in range(2):
                                rhs = xpad[kh][:, 1 + h0:1 + h0 + CH, 1:1 + W]
                                nc.tensor.matmul(pp[:].rearrange("p (h w) -> p h w", w=W),
                                                 w2_lhs(kh, h), rhs,
                                                 start=(kh == 0), stop=(kh == 1))
                            s = epool.tile([CS, NCOL], f32, tag=f"es2{h}")
                            nc.scalar.activation(s[:], pp[:], SILU,
                                                 bias=bn_bias(2, h),
                                                 scale=bn_scale(2, h))
                            es[(2, h)] = s
                        # weighted combine on DVE, then store
                        for h in range(2):
                            acc = apool.tile([CS, NCOL], f32, tag=f"acc{h}")
                            nc.vector.tensor_scalar_mul(acc[:], es[(2, h)][:],
                                                        coeff(b, 2))
                            nc.vector.scalar_tensor_tensor(
                                acc[:], es[(0, h)][:], coeff(b, 0), acc[:], MUL, ADD)
                            nc.vector.scalar_tensor_tensor(
                                acc[:], es[(1, h)][:], coeff(b, 1), acc[:], MUL, ADD)
                            nc.sync.dma_start(
                                out_d.ap()[b, h * CS:(h + 1) * CS,
                                           h0 * W:(h0 + CH) * W], acc[:])
    nc.finalize()
    return nc


def _prep(inputs):
    """Host-side preprocessing -> per-core input maps."""
    x = np.ascontiguousarray(np.asarray(inputs["x"], dtype=np.float32))
    weights = np.asarray(inputs["weights"], dtype=np.float32)
    indices = np.asarray(inputs["indices"])

    # routing coefficients  [B, 3]
    coeff = np.zeros((B, 3), np.float32)
    for e in range(3):
        coeff[:, e] = (weights * (indices == e)).sum(axis=1)

    def fold_bn(s, bvec, m, v):
        inv = s / np.sqrt(v + BN_EPS)
        return inv.astype(np.float32), (bvec - m * inv).astype(np.float32)

    sc0, bi0 = fold_bn(*(np.asarray(inputs[k], np.float32)
                         for k in ("bn0_s", "bn0_b", "bn0_m", "bn0_v")))
    sc1, bi1 = fold_bn(*(np.asarray(inputs[k], np.float32)
                         for k in ("bn1_s", "bn1_b", "bn1_m", "bn1_v")))
    sc2, bi2 = fold_bn(*(np.asarray(inputs[k], np.float32)
                         for k in ("bn2_s", "bn2_b", "bn2_m", "bn2_v")))

    dw0 = np.asarray(inputs["dw_w0"], np.float32).reshape(CS, 9)
    dw1 = np.asarray(inputs["dw_w1"], np.float32).reshape(CS, 9)
    pw0 = np.asarray(inputs["pw_w0"], np.float32).reshape(OUT, CS)
    pw1 = np.asarray(inputs["pw_w1"], np.float32).reshape(OUT, CS)
    w2 = np.asarray(inputs["w2"], np.float32).reshape(OUT, C)

    wr = np.zeros((CS, NWR), np.float32)
    ar = np.arange(CS)
    for e, dwk in enumerate((dw0, dw1)):
        for t in range(9):
            wr[ar, DW_OFF + (e * 9 + t) * CS + ar] = dwk[:, t]
    wr[:, PW_OFF:PW_OFF + 256] = pw0.T
    wr[:, PW_OFF + 256:PW_OFF + 512] = pw1.T
    wr[:, W2_OFF:W2_OFF + 256] = w2[:, :CS].T
    wr[:, W2_OFF + 256:W2_OFF + 512] = w2[:, CS:].T

    # padded input, [B, 2, CS, HP, WP]
    xpad = np.zeros((B, 2, CS, HP, WP), np.float32)
    xr = x.reshape(B, 2, CS, H, W)
    xpad[:, :, :, 1:-1, 1:-1] = xr

    in_maps = []
    for c in range(NCORES):
        bs = slice(c * BL, (c + 1) * BL)
        csb = np.zeros((CS, 24), np.float32)
        csb[:, 0] = sc0[:CS];  csb[:, 1] = sc0[CS:]
        csb[:, 2] = sc1[:CS];  csb[:, 3] = sc1[CS:]
        csb[:, 4] = sc2[:CS];  csb[:, 5] = sc2[CS:]
        csb[:, 6] = bi0[:CS];  csb[:, 7] = bi0[CS:]
        csb[:, 8] = bi1[:CS];  csb[:, 9] = bi1[CS:]
        csb[:, 10] = bi2[:CS]; csb[:, 11] = bi2[CS:]
        for bl in range(BL):
            for e in range(3):
                csb[:, 12 + 3 * bl + e] = coeff[c * BL + bl, e]
        in_maps.append({
            "xp": np.ascontiguousarray(
                xpad[bs].reshape(BL * 2, CS, SP)),
            "wr": wr,
            "cs": csb,
        })
    return in_maps


def kernel(**inputs) -> np.ndarray:
    from concourse.bass_utils import run_bass_kernel_spmd

    if "nc" not in _cache:
        _cache["nc"] = _build_program()
    nc = _cache["nc"]

    in_maps = _prep(inputs)
    res = run_bass_kernel_spmd(nc, in_maps, core_ids=list(range(NCORES)))
    out = np.concatenate([res.results[c]["out"] for c in range(NCORES)], axis=0)
    return out.reshape(B, OUT, H, W).astype(np.float32)


# revision 4
# speedup vs baseline: 1.2159x; 1.2159x over previous
"""Trainium2 Bass kernel for nn_DualModalExpertContainer.

Strategy (8 NeuronCores, data-parallel over batch, 4 batches/core):
  - expert0/1 depthwise 3x3: 9 accumulating fp32r diagonal matmuls on the
    TensorEngine over a host-padded 66x66 image (N=512 chunks, PSUM accum).
  - pointwise convs (128->256) and expert2 1x1 (256->256): fp32r matmuls.
  - BN+SiLU fused on the Scalar engine (per-partition scale/bias APs).
  - routing combine sum_e coeff[b,e]*silu_e on the Vector engine via
    tensor_scalar + 2 fused scalar_tensor_tensor ops.
  - host precomputes: padded x, folded BN scale/bias, routing coeffs,
    transposed/diagonal weight tiles.
"""
import sys

sys.path.insert(0, '/opt/trn_rl_repo')

import numpy as np

BN_EPS = 1e-5
B, C, H, W, OUT = 32, 256, 64, 64, 256
CS = C // 2                      # 128 channels per expert half
NCORES = 8
BL = B // NCORES                 # 4 batches per core
HP, WP = H + 2, W + 2            # 66x66 padded
SP = HP * WP                     # 4356
S = H * W                        # 4096
CH = 8                           # h-rows per chunk
NCK = H // CH                    # 8 chunks per image
NCOL = CH * W                    # 512 columns per chunk

# packed fp32r weight tile column offsets
DW_OFF = 0                       # 2 experts * 9 taps * 128 = 2304
PW_OFF = 2304                    # 2 experts * 256 = 512
W2_OFF = 2816                    # 2 k-halves * 256 = 512
NWR = 3328

_cache = {}


def _build_program():
    import concourse.bass as bass
    import concourse.mybir as mybir
    from concourse.bacc import Bacc
    from concourse import tile

    f32 = mybir.dt.float32
    f32r = mybir.dt.float32r
    SILU = mybir.ActivationFunctionType.Silu
    MUL = mybir.AluOpType.mult
    ADD = mybir.AluOpType.add

    nc = Bacc()
    xp_in = nc.declare_dram_parameter("xp", [BL * 2, CS, SP], f32r, isOutput=False)
    wr_in = nc.declare_dram_parameter("wr", [CS, NWR], f32r, isOutput=False)
    cs_in = nc.declare_dram_parameter("cs", [CS, 24], f32, isOutput=False)
    out_d = nc.declare_dram_parameter("out", [BL, OUT, S], f32, isOutput=True)

    with tile.TileContext(nc) as tc:
        with (
            tc.tile_pool(name="const", bufs=1) as cpool,
            tc.tile_pool(name="xpad", bufs=4) as xpool,
            tc.tile_pool(name="ysb", bufs=2) as ypool,
            tc.tile_pool(name="esb", bufs=2) as epool,
            tc.tile_pool(name="accs", bufs=3) as apool,
            tc.tile_pool(name="psy", bufs=2, space="PSUM") as psy_pool,
            tc.tile_pool(name="pse", bufs=1, space="PSUM") as pse_pool,
        ):
            wr = cpool.tile([CS, NWR], f32r)
            nc.sync.dma_start(wr[:], wr_in.ap())
            cs = cpool.tile([CS, 24], f32)
            nc.sync.dma_start(cs[:], cs_in.ap())

            def dw_lhs(e, t):
                off = DW_OFF + (e * 9 + t) * CS
                return wr[:, off:off + CS]

            def pw_lhs(e, h):
                off = PW_OFF + e * 256 + h * CS
                return wr[:, off:off + CS]

            def w2_lhs(kh, h):
                off = W2_OFF + kh * 256 + h * CS
                return wr[:, off:off + CS]

            def bn_scale(e, h):
                return cs[:, 2 * e + h:2 * e + h + 1]

            def bn_bias(e, h):
                return cs[:, 6 + 2 * e + h:6 + 2 * e + h + 1]

            def coeff(b, e):
                return cs[:, 12 + 3 * b + e:12 + 3 * b + e + 1]

            for b in range(BL):
                xpad = [None, None]
                for e in range(2):
                    t = xpool.tile([CS, HP, WP], f32r, tag=f"xp{e}")
                    nc.sync.dma_start(
                        t[:].rearrange("p h w -> p (h w)"), xp_in.ap()[2 * b + e])
                    xpad[e] = t

                # software-pipelined: DW for chunk n overlaps PW/e2/silu/
                # combine for chunk n-1
                ysb = {}
                for n in range(NCK + 1):
                    if n < NCK:
                        h0 = n * CH
                        for e in range(2):
                            ps = psy_pool.tile([CS, NCOL], f32, tag=f"y{e}")
                            psv = ps[:].rearrange("p (h w) -> p h w", w=W)
                            for t in range(9):
                                dh, dw = t // 3, t % 3
                                rhs = xpad[e][:, h0 + dh:h0 + dh + CH, dw:dw + W]
                                nc.tensor.matmul(psv, dw_lhs(e, t), rhs,
                                                 start=(t == 0), stop=(t == 8))
                            y = ypool.tile([CS, NCOL], f32r, tag=f"ysb{e}")
                            nc.scalar.copy(y[:], ps[:])
                            ysb[(n, e)] = y
                    if n >= 1:
                        m = n - 1
                        h0 = m * CH
                        es = {}
                        # pointwise experts 0/1
                        for e in range(2):
                            yt = ysb.pop((m, e))
                            for h in range(2):
                                pp = pse_pool.tile([CS, NCOL], f32, tag=f"pw{h}")
                                nc.tensor.matmul(pp[:], pw_lhs(e, h), yt[:],
                                                 start=True, stop=True)
                                s = epool.tile([CS, NCOL], f32, tag=f"es{e}{h}")
                                nc.scalar.activation(s[:], pp[:], SILU,
                                                     bias=bn_bias(e, h),
                                                     scale=bn_scale(e, h))
                                es[(e, h)] = s
                        # expert 2: 1x1 over all 256 channels
                        for h in range(2):
                            pp = pse_pool.tile([CS, NCOL], f32, tag=f"e2{h}")
                            for kh in range(2):
                                rhs = xpad[kh][:, 1 + h0:1 + h0 + CH, 1:1 + W]
                                nc.tensor.matmul(pp[:].rearrange("p (h w) -> p h w", w=W),
                                                 w2_lhs(kh, h), rhs,
                                                 start=(kh == 0), stop=(kh == 1))
                            s = epool.tile([CS, NCOL], f32, tag=f"es2{h}")
                            nc.scalar.activation(s[:], pp[:], SILU,
                                                 bias=bn_bias(2, h),
                                                 scale=bn_scale(2, h))
                            es[(2, h)] = s
                        # weighted combine on DVE, then store
                        for h in range(2):
                            acc = apool.tile([CS, NCOL], f32, tag=f"acc{h}")
                            nc.vector.tensor_scalar_mul(acc[:], es[(2, h)][:],
                                                        coeff(b, 2))
                            nc.vector.scalar_tensor_tensor(
                                acc[:], es[(0, h)][:], coeff(b, 0), acc[:], MUL, ADD)
                            nc.vector.scalar_tensor_tensor(
                                acc[:], es[(1, h)][:], coeff(b, 1), acc[:], MUL, ADD)
                            nc.sync.dma_start(
                                out_d.ap()[b, h * CS:(h + 1) * CS,
                                           h0 * W:(h0 + CH) * W], acc[:])
    nc.finalize()
    return nc


def _prep(inputs):
    """Host-side preprocessing -> per-core input maps."""
    x = np.ascontiguousarray(np.asarray(inputs["x"], dtype=np.float32))
    weights = np.asarray(inputs["weights"], dtype=np.float32)
    indices = np.asarray(inputs["indices"])

    # routing coefficients  [B, 3]
    coeff = np.zeros((B, 3), np.float32)
    for e in range(3):
        coeff[:, e] = (weights * (indices == e)).sum(axis=1)

    def fold_bn(s, bvec, m, v):
        inv = s / np.sqrt(v + BN_EPS)
        return inv.astype(np.float32), (bvec - m * inv).astype(np.float32)

    sc0, bi0 = fold_bn(*(np.asarray(inputs[k], np.float32)
                         for k in ("bn0_s", "bn0_b", "bn0_m", "bn0_v")))
    sc1, bi1 = fold_bn(*(np.asarray(inputs[k], np.float32)
                         for k in ("bn1_s", "bn1_b", "bn1_m", "bn1_v")))
    sc2, bi2 = fold_bn(*(np.asarray(inputs[k], np.float32)
                         for k in ("bn2_s", "bn2_b", "bn2_m", "bn2_v")))

    dw0 = np.asarray(inputs["dw_w0"], np.float32).reshape(CS, 9)
    dw1 = np.asarray(inputs["dw_w1"], np.float32).reshape(CS, 9)
    pw0 = np.asarray(inputs["pw_w0"], np.float32).reshape(OUT, CS)
    pw1 = np.asarray(inputs["pw_w1"], np.float32).reshape(OUT, CS)
    w2 = np.asarray(inputs["w2"], np.float32).reshape(OUT, C)

    wr = np.zeros((CS, NWR), np.float32)
    ar = np.arange(CS)
    for e, dwk in enumerate((dw0, dw1)):
        for t in range(9):
            wr[ar, DW_OFF + (e * 9 + t) * CS + ar] = dwk[:, t]
    wr[:, PW_OFF:PW_OFF + 256] = pw0.T
    wr[:, PW_OFF + 256:PW_OFF + 512] = pw1.T
    wr[:, W2_OFF:W2_OFF + 256] = w2[:, :CS].T
    wr[:, W2_OFF + 256:W2_OFF + 512] = w2[:, CS:].T

    # padded input, [B, 2, CS, HP, WP]
    xpad = np.zeros((B, 2, CS, HP, WP), np.float32)
    xr = x.reshape(B, 2, CS, H, W)
    xpad[:, :, :, 1:-1, 1:-1] = xr

    in_maps = []
    for c in range(NCORES):
        bs = slice(c * BL, (c + 1) * BL)
        csb = np.zeros((CS, 24), np.float32)
        csb[:, 0] = sc0[:CS];  csb[:, 1] = sc0[CS:]
        csb[:, 2] = sc1[:CS];  csb[:, 3] = sc1[CS:]
        csb[:, 4] = sc2[:CS];  csb[:, 5] = sc2[CS:]
        csb[:, 6] = bi0[:CS];  csb[:, 7] = bi0[CS:]
        csb[:, 8] = bi1[:CS];  csb[:, 9] = bi1[CS:]
        csb[:, 10] = bi2[:CS]; csb[:, 11] = bi2[CS:]
        for bl in range(BL):
            for e in range(3):
                csb[:, 12 + 3 * bl + e] = coeff[c * BL + bl, e]
        in_maps.append({
            "xp": np.ascontiguousarray(
                xpad[bs].reshape(BL * 2, CS, SP)),
            "wr": wr,
            "cs": csb,
        })
    return in_maps


def _build_runner():
    """Jit-once runner over 8 cores (mirrors bass2jax.run_bass_via_pjrt)."""
    import jax
    import jax.numpy as jnp
    from jax.sharding import Mesh, PartitionSpec
    from jax.experimental.shard_map import shard_map
    import concourse.mybir as mybir
    import concourse.bass2jax as b2j

    nc = _build_program()
    b2j.install_neuronx_cc_hook()

    part_name = nc.partition_id_tensor.name if nc.partition_id_tensor else None
    in_names, out_names, out_avals = [], [], []
    for alloc in nc.m.functions[0].allocations:
        if not isinstance(alloc, mybir.MemoryLocationSet):
            continue
        name = alloc.memorylocations[0].name
        if alloc.kind == "ExternalInput":
            if name != part_name:
                in_names.append(name)
        elif alloc.kind == "ExternalOutput":
            out_names.append(name)
            out_avals.append(jax.core.ShapedArray(
                tuple(alloc.tensor_shape), mybir.dt.np(alloc.dtype)))
    n_params = len(in_names)
    all_names = in_names + out_names
    if part_name is not None:
        all_names = all_names + [part_name]

    def _body(*args):
        operands = list(args)
        if part_name is not None:
            operands.append(b2j.partition_id_tensor())
        return tuple(b2j._bass_exec_p.bind(
            *operands,
            out_avals=tuple(out_avals),
            in_names=tuple(all_names),
            out_names=tuple(out_names),
            lowering_input_output_aliases=(),
            sim_require_finite=True,
            sim_require_nnan=True,
            nc=nc,
        ))

    devices = jax.devices()[:NCORES]
    mesh = Mesh(np.asarray(devices), ("core",))
    n_outs = len(out_names)
    donate = tuple(range(n_params, n_params + n_outs))
    sharded = jax.jit(
        shard_map(_body, mesh=mesh,
                  in_specs=(PartitionSpec("core"),) * (n_params + n_outs),
                  out_specs=(PartitionSpec("core"),) * n_outs,
                  check_rep=False),
        donate_argnums=donate, keep_unused=True)

    out_shapes = [(NCORES * a.shape[0], *a.shape[1:]) for a in out_avals]
    out_dtypes = [a.dtype for a in out_avals]

    def run(in_maps):
        concat_in = [np.concatenate([m[n] for m in in_maps], axis=0)
                     for n in in_names]
        zeros = [jnp.zeros(s, d) for s, d in zip(out_shapes, out_dtypes)]
        outs = sharded(*concat_in, *zeros)
        return [np.asarray(o) for o in outs], out_names

    return run


def kernel(**inputs) -> np.ndarray:
    if "runner" not in _cache:
        _cache["runner"] = _build_runner()
    in_maps = _prep(inputs)
    outs, out_names = _cache["runner"](in_maps)
    out = outs[out_names.index("out")]
    return out.reshape(B, OUT, H, W).astype(np.float32)
